# revision 2
# baseline (speedup 1.0000x reference)
"""LoRA linear kernel for 8 TRN2 NeuronCores — fp16 host-cast.

out = x @ (base_weight + SCALE * lora_B @ lora_A).T + bias
for x [4, 2048, 4096], base_weight [4096, 4096], rank 8.

Sharding ('r2c4'): 2 token-halves x 4 d_out-quarters = 8 cores
(tensor-parallel on d_out per the hint, plus a token split that keeps
per-core x traffic low and W' SBUF-resident at 32 KB/partition).

Host prep: W' = W + SCALE*B@A is computed in numpy (268 MFLOP) and both
x and W'.T are cast to fp16 and pre-tiled, so HBM traffic halves vs f32
(32 MB x + 8 MB W' + 16 MB f32 out per core) and there is no device-side
LoRA prep phase. fp16 operand rounding gives ~3e-4 rel L2 error.

Per core: W' lives in SBUF as 32 [128, 1024] fp16 k-tiles. For each of
32 128-token tiles: one 1 MB contiguous DMA loads the pre-tiled x.T
block; 32x2 accumulating [128k,128t]x[128k,512o] matmuls (x k-tile
stationary, W' moving) fill 2 PSUM banks; the DVE adds bias during
PSUM->SBUF copyback; out rows DMA back contiguously. The first INTRO=4
token tiles' x DMAs are issued BEFORE the 8 MB W' preload (early-x) so
the PE's first matmuls aren't queued behind the W stream — measured
~10 us better cold-pass time; INTRO tiles are interleaved k-major so
the PE consumes W' k-tiles as they stream in.

Measured floor analysis (same-process A/B, slope over an on-device
For_i repeat loop, 8 cores concurrent; micro-benchmarks in exp_micro.py):
the N=512 fp16 matmul costs ~223 ns back-to-back on 1 core (2.4 GHz)
and ~266-270 ns with all 8 cores under sustained load (chip power
throttle, PE ~2.0 GHz: 512/2.0 + overhead). LDWEIGHTS is ALREADY FULLY
HIDDEN — explicit nc.tensor.ldweights + matmul(ldweights=False) with
1/2/4/8 matmuls per load, same-PSUM-bank accumulation runs, and oc-outer
ordering all measure the same per-MM cost (within noise), and this
kernel measures ratio 1.011 vs the same-process 8-core pure-PE floor
(2048 MMs x per-MM). The 2048 matmuls/core are minimal (stationary
128x128 and moving 512 both maxed), so steady state is at the hardware
floor; absolute pass time floats 540-580 us with device state.

Probed and rejected: explicit-LDW amortization (no effect, already
hidden), 1024-wide moving operand (walrus codegen rejects), 3D-AP
matmul splitting (ISA check rejects), fp8 e4m3 raw (3.75e-2 rel err vs
the 2e-2 gate, incl. rescaling variants), fp8 DoubleRow with error
compensation (correction passes cost >= the fp16 saving), fp16 output
(neutral), r4c2/r8c1 shardings (equal-to-worse; r8c1 overflows SBUF).
Device-state variance is +-10% across sessions — only same-process A/B
comparisons are meaningful.
"""
import sys

if '/opt/trn_rl_repo' not in sys.path:
    sys.path.insert(0, '/opt/trn_rl_repo')

from contextlib import ExitStack

import numpy as np

import concourse.bacc as bacc
import concourse.mybir as mybir
import concourse.tile as tile
from concourse.bass_utils import run_bass_kernel_spmd

SCALE = 16.0 / 8.0  # alpha / rank

P = 128
K = 4096           # d_in (contraction)
KT = K // P        # 32 k-tiles
D_OUT = 4096
B, S = 4, 2048
T_FULL = B * S     # 8192 tokens
N_CORES = 8

MODE = 'r2c4'
# mode: (r_split, c_split, mm_width)
MODES = {'r2c4': (2, 4, 512), 'r4c2': (4, 2, 512)}

_nc_cache = {}


def _dims(mode):
    r_split, c_split, mmw = MODES[mode]
    t_core = T_FULL // r_split
    tt = t_core // P
    o_core = D_OUT // c_split
    oc = o_core // mmw
    return r_split, c_split, t_core, tt, o_core, oc, mmw


def build_nc(repeat=1, mode=None):
    mode = mode or MODE
    key = (mode, repeat)
    if key in _nc_cache:
        return _nc_cache[key]
    f32 = mybir.dt.float32
    f16 = mybir.dt.float16
    _, _, t_core, TT, O_CORE, OC, MMW = _dims(mode)

    nc = bacc.Bacc(None, target_bir_lowering=False)
    # x blocks: [t_tile, p(k-within-tile), kt, j(token-within-tile)]
    xb = nc.dram_tensor("xb", [TT, P, KT, P], f16, kind="ExternalInput")
    wt = nc.dram_tensor("wt", [KT, P, O_CORE], f16, kind="ExternalInput")
    biasb = nc.dram_tensor("biasb", [P, O_CORE], f32, kind="ExternalInput")
    out = nc.dram_tensor("out", [t_core, O_CORE], f32, kind="ExternalOutput")

    with ExitStack() as ctx:
        tc = ctx.enter_context(tile.TileContext(nc))
        wpool = ctx.enter_context(tc.tile_pool(name="wpool", bufs=1))
        cpool = ctx.enter_context(tc.tile_pool(name="cpool", bufs=1))
        # PSUM: 8 banks of 512 f32; each psum tile spans MMW/512 banks.
        banks_per_tile = OC * (MMW // 512)
        psbufs = max(2, 8 // banks_per_tile)
        nb = 2 if O_CORE == 2048 else 6
        xpool = ctx.enter_context(tc.tile_pool(name="xpool", bufs=nb))
        opool = ctx.enter_context(tc.tile_pool(name="opool", bufs=nb))
        pspool = ctx.enter_context(tc.tile_pool(name="ps", bufs=psbufs,
                                                space="PSUM"))
        INTRO = max(1, min(TT, psbufs))
        # early-x: dedicated pool — these buffers must never rotate into
        # the main loop (a For_i body reuses them by reference each pass)
        prepool = ctx.enter_context(tc.tile_pool(name="prex", bufs=INTRO))

        bias_t = cpool.tile([P, O_CORE], f32, tag="bias", name="bias_t")
        nc.sync.dma_start(bias_t[:], biasb[:])

        # ---- first INTRO x tiles queued ahead of the W' preload ----
        pre_x = []
        for tt in range(INTRO):
            xt = prepool.tile([P, KT, P], f16, name=f"prex_{tt}", tag="prex")
            nc.sync.dma_start(xt[:], xb[tt])
            pre_x.append(xt)

        # ---- W' tiles resident in SBUF as [k, o], fp16 ----
        wtiles = []
        for k in range(KT):
            w_t = wpool.tile([P, O_CORE], f16, tag=f"w{k}", name=f"w_{k}")
            nc.sync.dma_start(w_t[:], wt[k])
            wtiles.append(w_t)

        # ---- main loop: out[t, o] = x_tile.T @ W' (+ bias) ----
        def load_x(tt):
            xt = xpool.tile([P, KT, P], f16, name=f"xt_{tt}", tag="xt")
            nc.sync.dma_start(xt[:], xb[tt])
            return xt

        def alloc_ps(tt):
            return [pspool.tile([P, MMW], f32, tag=f"ps{oc}",
                                name=f"ps_{tt}_{oc}")
                    for oc in range(OC)]

        def flush(tt, pss):
            o_t = opool.tile([P, O_CORE], f32, name=f"ot_{tt}", tag="ot")
            for oc in range(OC):
                sl = slice(oc * MMW, (oc + 1) * MMW)
                nc.vector.tensor_add(o_t[:, sl], pss[oc][:], bias_t[:, sl])
            nc.sync.dma_start(out[tt * P:(tt + 1) * P, :], o_t[:])

        def mms(xt, pss, k):
            for oc in range(OC):
                nc.tensor.matmul(
                    pss[oc][:],
                    xt[:, k, :],
                    wtiles[k][:, oc * MMW:(oc + 1) * MMW],
                    start=(k == 0), stop=(k == KT - 1),
                )

        def main_pass():
            ixt = pre_x[:INTRO]
            ips = [alloc_ps(tt) for tt in range(INTRO)]
            for k in range(KT):
                for tt in range(INTRO):
                    mms(ixt[tt], ips[tt], k)
            for tt in range(INTRO):
                flush(tt, ips[tt])
            for tt in range(INTRO, TT):
                xt = load_x(tt)
                pss = alloc_ps(tt)
                for k in range(KT):
                    mms(xt, pss, k)
                flush(tt, pss)

        if repeat == 1:
            main_pass()
        else:
            with tc.For_i(0, repeat, 1):
                main_pass()

    nc.compile()
    _nc_cache[key] = nc
    return nc


def _prep_in_maps(x, base_weight, lora_A, lora_B, bias, mode=None):
    mode = mode or MODE
    r_split, c_split, t_core, TT, O_CORE, OC, MMW = _dims(mode)
    w_full = base_weight.astype(np.float32) + \
        SCALE * (lora_B.astype(np.float32) @ lora_A.astype(np.float32))
    WT = np.ascontiguousarray(w_full.T).astype(np.float16)  # [k, o]
    x2d = x.reshape(T_FULL, K).astype(np.float16)
    bias = bias.astype(np.float32, copy=False)

    xbs = []
    for h in range(r_split):
        xh = x2d[h * t_core:(h + 1) * t_core]
        # [tt, j(tok), kt, p(k)] -> [tt, p, kt, j]
        xb = np.ascontiguousarray(
            xh.reshape(TT, P, KT, P).transpose(0, 3, 2, 1))
        xbs.append(xb)

    in_maps = []
    for h in range(r_split):
        for q in range(c_split):
            osl = slice(q * O_CORE, (q + 1) * O_CORE)
            wtq = np.ascontiguousarray(
                WT[:, osl].reshape(KT, P, O_CORE))
            biasb = np.ascontiguousarray(
                np.broadcast_to(bias[osl][None, :], (P, O_CORE)))
            in_maps.append({"xb": xbs[h], "wt": wtq, "biasb": biasb})
    return in_maps


def _assemble(results, mode=None):
    mode = mode or MODE
    r_split, c_split, t_core, TT, O_CORE, OC, MMW = _dims(mode)
    flat = np.empty((T_FULL, D_OUT), dtype=np.float32)
    i = 0
    for h in range(r_split):
        for q in range(c_split):
            flat[h * t_core:(h + 1) * t_core,
                 q * O_CORE:(q + 1) * O_CORE] = results[i]["out"]
            i += 1
    return flat.reshape(B, S, D_OUT)


def kernel(x, base_weight, lora_A, lora_B, bias):
    x = np.asarray(x)
    base_weight = np.asarray(base_weight)
    lora_A = np.asarray(lora_A)
    lora_B = np.asarray(lora_B)
    bias = np.asarray(bias)
    nc = build_nc()
    in_maps = _prep_in_maps(x, base_weight, lora_A, lora_B, bias)
    res = run_bass_kernel_spmd(nc, in_maps, core_ids=list(range(N_CORES)))
    return _assemble(res.results)


# revision 4
# speedup vs baseline: 1.0227x; 1.0227x over previous
"""LoRA linear kernel for 8 TRN2 NeuronCores — fp16 host-cast.

out = x @ (base_weight + SCALE * lora_B @ lora_A).T + bias
for x [4, 2048, 4096], base_weight [4096, 4096], rank 8.

Sharding ('r2c4'): 2 token-halves x 4 d_out-quarters = 8 cores
(tensor-parallel on d_out per the hint, plus a token split that keeps
per-core x traffic low and W' SBUF-resident at 32 KB/partition).

Host prep: W' = W + SCALE*B@A is computed in numpy (268 MFLOP) and both
x and W'.T are cast to fp16 and pre-tiled, so HBM traffic halves vs f32
(32 MB x + 8 MB W' + 16 MB f32 out per core) and there is no device-side
LoRA prep phase. fp16 operand rounding gives ~3e-4 rel L2 error.

Per core: W' lives in SBUF as 32 [128, 1024] fp16 k-tiles. For each of
32 128-token tiles: one 1 MB contiguous DMA loads the pre-tiled x.T
block; 32x2 accumulating [128k,128t]x[128k,512o] matmuls (x k-tile
stationary, W' moving) fill 2 PSUM banks; the DVE adds bias during
PSUM->SBUF copyback; out rows DMA back contiguously. The first INTRO=4
token tiles' x DMAs are interleaved ahead of the early W' k-tile DMAs
(bias, x0,w0, x1,w1, ..., w31) so the PE's first matmuls aren't queued
behind the 8 MB W' preload — measured ~18-27 us better cold-pass time
than W-first emission; INTRO tiles are interleaved k-major so the PE
consumes W' k-tiles as they stream in.

Measured floor analysis (same-process A/B, slope over an on-device
For_i repeat loop, 8 cores concurrent; micro-benchmarks in exp_micro.py):
the N=512 fp16 matmul costs ~223 ns back-to-back on 1 core (2.4 GHz)
and ~266-270 ns with all 8 cores under sustained load (chip power
throttle, PE ~2.0 GHz: 512/2.0 + overhead). LDWEIGHTS is ALREADY FULLY
HIDDEN — explicit nc.tensor.ldweights + matmul(ldweights=False) with
1/2/4/8 matmuls per load, same-PSUM-bank accumulation runs, and oc-outer
ordering all measure the same per-MM cost (within noise), and this
kernel measures ratio 1.011 vs the same-process 8-core pure-PE floor
(2048 MMs x per-MM). The 2048 matmuls/core are minimal (stationary
128x128 and moving 512 both maxed), so steady state is at the hardware
floor; absolute pass time floats 540-580 us with device state.

Probed and rejected: explicit-LDW amortization (no effect, already
hidden), 1024-wide moving operand (walrus codegen rejects), 3D-AP
matmul splitting (ISA check rejects), fp8 e4m3 raw (3.75e-2 rel err vs
the 2e-2 gate, incl. rescaling variants), fp8 DoubleRow with error
compensation (correction passes cost >= the fp16 saving), fp16 output
(neutral), r4c2/r8c1 shardings (equal-to-worse; r8c1 overflows SBUF).
Device-state variance is +-10% across sessions — only same-process A/B
comparisons are meaningful.
"""
import sys

if '/opt/trn_rl_repo' not in sys.path:
    sys.path.insert(0, '/opt/trn_rl_repo')

from contextlib import ExitStack

import numpy as np

import concourse.bacc as bacc
import concourse.mybir as mybir
import concourse.tile as tile
from concourse.bass_utils import run_bass_kernel_spmd

SCALE = 16.0 / 8.0  # alpha / rank

P = 128
K = 4096           # d_in (contraction)
KT = K // P        # 32 k-tiles
D_OUT = 4096
B, S = 4, 2048
T_FULL = B * S     # 8192 tokens
N_CORES = 8

MODE = 'r2c4'
# mode: (r_split, c_split, mm_width)
MODES = {'r2c4': (2, 4, 512), 'r4c2': (4, 2, 512)}

_nc_cache = {}


def _dims(mode):
    r_split, c_split, mmw = MODES[mode]
    t_core = T_FULL // r_split
    tt = t_core // P
    o_core = D_OUT // c_split
    oc = o_core // mmw
    return r_split, c_split, t_core, tt, o_core, oc, mmw


def build_nc(repeat=1, mode=None):
    mode = mode or MODE
    key = (mode, repeat)
    if key in _nc_cache:
        return _nc_cache[key]
    f32 = mybir.dt.float32
    f16 = mybir.dt.float16
    _, _, t_core, TT, O_CORE, OC, MMW = _dims(mode)

    nc = bacc.Bacc(None, target_bir_lowering=False)
    # x blocks: [t_tile, p(k-within-tile), kt, j(token-within-tile)]
    xb = nc.dram_tensor("xb", [TT, P, KT, P], f16, kind="ExternalInput")
    wt = nc.dram_tensor("wt", [KT, P, O_CORE], f16, kind="ExternalInput")
    biasb = nc.dram_tensor("biasb", [P, O_CORE], f32, kind="ExternalInput")
    out = nc.dram_tensor("out", [t_core, O_CORE], f32, kind="ExternalOutput")

    with ExitStack() as ctx:
        tc = ctx.enter_context(tile.TileContext(nc))
        wpool = ctx.enter_context(tc.tile_pool(name="wpool", bufs=1))
        cpool = ctx.enter_context(tc.tile_pool(name="cpool", bufs=1))
        # PSUM: 8 banks of 512 f32; each psum tile spans MMW/512 banks.
        banks_per_tile = OC * (MMW // 512)
        psbufs = max(2, 8 // banks_per_tile)
        nb = 2 if O_CORE == 2048 else 6
        xpool = ctx.enter_context(tc.tile_pool(name="xpool", bufs=nb))
        opool = ctx.enter_context(tc.tile_pool(name="opool", bufs=nb))
        pspool = ctx.enter_context(tc.tile_pool(name="ps", bufs=psbufs,
                                                space="PSUM"))
        INTRO = max(1, min(TT, psbufs))
        # early-x: dedicated pool — these buffers must never rotate into
        # the main loop (a For_i body reuses them by reference each pass)
        prepool = ctx.enter_context(tc.tile_pool(name="prex", bufs=INTRO))

        bias_t = cpool.tile([P, O_CORE], f32, tag="bias", name="bias_t")
        nc.sync.dma_start(bias_t[:], biasb[:])

        # ---- W' preload with the first INTRO x-tile DMAs interleaved
        # ahead of each early W k-tile (x0,w0,x1,w1,...), so the PE's first
        # matmuls aren't queued behind the 8 MB W stream ----
        pre_x = []
        wtiles = []
        for k in range(KT):
            if k < INTRO:
                xt = prepool.tile([P, KT, P], f16, name=f"prex_{k}",
                                  tag="prex")
                nc.sync.dma_start(xt[:], xb[k])
                pre_x.append(xt)
            w_t = wpool.tile([P, O_CORE], f16, tag=f"w{k}", name=f"w_{k}")
            nc.sync.dma_start(w_t[:], wt[k])
            wtiles.append(w_t)

        # ---- main loop: out[t, o] = x_tile.T @ W' (+ bias) ----
        def load_x(tt):
            xt = xpool.tile([P, KT, P], f16, name=f"xt_{tt}", tag="xt")
            nc.sync.dma_start(xt[:], xb[tt])
            return xt

        def alloc_ps(tt):
            return [pspool.tile([P, MMW], f32, tag=f"ps{oc}",
                                name=f"ps_{tt}_{oc}")
                    for oc in range(OC)]

        def flush(tt, pss):
            o_t = opool.tile([P, O_CORE], f32, name=f"ot_{tt}", tag="ot")
            for oc in range(OC):
                sl = slice(oc * MMW, (oc + 1) * MMW)
                nc.vector.tensor_add(o_t[:, sl], pss[oc][:], bias_t[:, sl])
            nc.sync.dma_start(out[tt * P:(tt + 1) * P, :], o_t[:])

        def mms(xt, pss, k):
            for oc in range(OC):
                nc.tensor.matmul(
                    pss[oc][:],
                    xt[:, k, :],
                    wtiles[k][:, oc * MMW:(oc + 1) * MMW],
                    start=(k == 0), stop=(k == KT - 1),
                )

        def main_pass():
            ixt = pre_x[:INTRO]
            ips = [alloc_ps(tt) for tt in range(INTRO)]
            for k in range(KT):
                for tt in range(INTRO):
                    mms(ixt[tt], ips[tt], k)
            for tt in range(INTRO):
                flush(tt, ips[tt])
            for tt in range(INTRO, TT):
                xt = load_x(tt)
                pss = alloc_ps(tt)
                for k in range(KT):
                    mms(xt, pss, k)
                flush(tt, pss)

        if repeat == 1:
            main_pass()
        else:
            with tc.For_i(0, repeat, 1):
                main_pass()

    nc.compile()
    _nc_cache[key] = nc
    return nc


def _prep_in_maps(x, base_weight, lora_A, lora_B, bias, mode=None):
    mode = mode or MODE
    r_split, c_split, t_core, TT, O_CORE, OC, MMW = _dims(mode)
    w_full = base_weight.astype(np.float32) + \
        SCALE * (lora_B.astype(np.float32) @ lora_A.astype(np.float32))
    WT = np.ascontiguousarray(w_full.T).astype(np.float16)  # [k, o]
    x2d = x.reshape(T_FULL, K).astype(np.float16)
    bias = bias.astype(np.float32, copy=False)

    xbs = []
    for h in range(r_split):
        xh = x2d[h * t_core:(h + 1) * t_core]
        # [tt, j(tok), kt, p(k)] -> [tt, p, kt, j]
        xb = np.ascontiguousarray(
            xh.reshape(TT, P, KT, P).transpose(0, 3, 2, 1))
        xbs.append(xb)

    in_maps = []
    for h in range(r_split):
        for q in range(c_split):
            osl = slice(q * O_CORE, (q + 1) * O_CORE)
            wtq = np.ascontiguousarray(
                WT[:, osl].reshape(KT, P, O_CORE))
            biasb = np.ascontiguousarray(
                np.broadcast_to(bias[osl][None, :], (P, O_CORE)))
            in_maps.append({"xb": xbs[h], "wt": wtq, "biasb": biasb})
    return in_maps


def _assemble(results, mode=None):
    mode = mode or MODE
    r_split, c_split, t_core, TT, O_CORE, OC, MMW = _dims(mode)
    flat = np.empty((T_FULL, D_OUT), dtype=np.float32)
    i = 0
    for h in range(r_split):
        for q in range(c_split):
            flat[h * t_core:(h + 1) * t_core,
                 q * O_CORE:(q + 1) * O_CORE] = results[i]["out"]
            i += 1
    return flat.reshape(B, S, D_OUT)


def kernel(x, base_weight, lora_A, lora_B, bias):
    x = np.asarray(x)
    base_weight = np.asarray(base_weight)
    lora_A = np.asarray(lora_A)
    lora_B = np.asarray(lora_B)
    bias = np.asarray(bias)
    nc = build_nc()
    in_maps = _prep_in_maps(x, base_weight, lora_A, lora_B, bias)
    res = run_bass_kernel_spmd(nc, in_maps, core_ids=list(range(N_CORES)))
    return _assemble(res.results)
